# revision 4
# baseline (speedup 1.0000x reference)
"""Causal self-attention (B=4, T=2048, H=1024, NH=16, HD=64) on 8 trn2 cores.

Sharding: tensor-parallel over heads — core c computes heads 2c and 2c+1 for
all batches. Q/K/V weights are column-sharded by head (host slices + pre-
transposes them); hidden_states is pre-transposed on host to X^T [H, B*T] so
the contraction dim (H) lands on SBUF partitions for every matmul.

Per-core dataflow (all matmuls fp32r = full-rate PE with ~1e-4 rounding):
  phase A (per batch b):
    X^T_b k-tiles [128, 2048] -> Q^T, K^T  [128=2*64 d, 2048 t]  (bias via ACT)
                              -> V^T [128, 2048] -> PE-transpose -> V_aug tiles
    V_aug[jt] = [V_nat[j, d_head] | ones]  (ones column makes the PV matmul
    also produce the softmax denominator as output row 64)
  phase B (per b, head h, query block ib of 512):
    S^T[j,i] = K^T_tile.T @ Q^T_block   (keys on partitions, queries free)
    P^T = exp(S^T/8 + causal + attn_mask[j])  (no max-subtraction: scores are
    O(1) by construction, exp stays in fp32 range)
    O^T[65, 512] += V_aug[jt].T @ P^T   (row 64 = denominator)
    normalize: recip(denom) -> gpsimd partition_broadcast -> DVE multiply
  output per core: O^T [4, 2, 64, 2048]; host transposes to [4, 2, 2048, 64]
  and concatenates heads.
"""

import numpy as np

B, T, H, NH = 4, 2048, 1024, 16
HD = H // NH  # 64
NCORES = 8
HPC = NH // NCORES  # heads per core = 2
BT = B * T

_CACHE = {}


def _build():
    from contextlib import ExitStack

    import concourse.bass as bass  # noqa: F401
    import concourse.mybir as mybir
    import concourse.tile as tile
    from concourse import bacc

    F32 = mybir.dt.float32
    F32R = mybir.dt.float32r

    nc = bacc.Bacc("TRN2", target_bir_lowering=False, num_devices=NCORES)

    xt = nc.declare_dram_parameter("xt", [H, BT], F32, isOutput=False)
    wqt = nc.declare_dram_parameter("wqt", [H, 128], F32, isOutput=False)
    wkt = nc.declare_dram_parameter("wkt", [H, 128], F32, isOutput=False)
    wvt = nc.declare_dram_parameter("wvt", [H, 128], F32, isOutput=False)
    bq = nc.declare_dram_parameter("bq", [128, 1], F32, isOutput=False)
    bk = nc.declare_dram_parameter("bk", [128, 1], F32, isOutput=False)
    bv = nc.declare_dram_parameter("bv", [128, 1], F32, isOutput=False)
    # amt[:, b*16+jt] = attention_mask[b, 0, 0, jt*128:(jt+1)*128]
    amt = nc.declare_dram_parameter("amt", [128, B * 16], F32, isOutput=False)
    out = nc.declare_dram_parameter("out", [B, HPC, HD, T], F32, isOutput=True)

    # Causal mask variants for diagonal-straddling [j-tile, i-block] pairs:
    # variant v masks (j + 128*v > i) within a [128, 512] tile.
    cm = np.zeros((128, 4, 512), dtype=np.float32)
    jj = np.arange(128)[:, None]
    ii = np.arange(512)[None, :]
    for v in range(4):
        cm[:, v, :] = np.where(jj + 128 * v > ii, -1e9, 0.0)
    cmask_dram = nc.inline_tensor(cm.reshape(128, 4 * 512), name="cmask")
    ident_dram = nc.inline_tensor(np.eye(128, dtype=np.float32), name="ident")
    ones_dram = nc.inline_tensor(np.ones((128, 1), dtype=np.float32), name="ones")

    NKT = H // 128  # 8 contraction tiles
    NIB = T // 512  # 4 query blocks
    NJT = T // 128  # 16 key tiles

    with tile.TileContext(nc) as tc:
        with ExitStack() as ctx:
            const = ctx.enter_context(tc.tile_pool(name="const", bufs=1))
            xpool = ctx.enter_context(tc.tile_pool(name="xpool", bufs=1))
            qkv = ctx.enter_context(tc.tile_pool(name="qkv", bufs=2))
            vapool = ctx.enter_context(tc.tile_pool(name="vapool", bufs=2))
            ppool = ctx.enter_context(tc.tile_pool(name="ppool", bufs=4))
            opool = ctx.enter_context(tc.tile_pool(name="opool", bufs=4))
            npool = ctx.enter_context(tc.tile_pool(name="npool", bufs=4))
            psA = ctx.enter_context(tc.tile_pool(name="psA", bufs=2, space="PSUM"))
            psS = ctx.enter_context(tc.tile_pool(name="psS", bufs=2, space="PSUM"))
            psT = ctx.enter_context(tc.tile_pool(name="psT", bufs=2, space="PSUM"))
            psO = ctx.enter_context(tc.tile_pool(name="psO", bufs=2, space="PSUM"))

            # --- constants / weights ---
            wt_sb = const.tile([128, 3 * H], F32R)
            for p, w in enumerate((wqt, wkt, wvt)):
                for kk in range(NKT):
                    nc.gpsimd.dma_start(
                        wt_sb[:, (p * NKT + kk) * 128 : (p * NKT + kk + 1) * 128],
                        w[kk * 128 : (kk + 1) * 128, :],
                    )
            bq_sb = const.tile([128, 1], F32)
            nc.sync.dma_start(bq_sb[:], bq[:])
            bk_sb = const.tile([128, 1], F32)
            nc.sync.dma_start(bk_sb[:], bk[:])
            bv_sb = const.tile([128, 1], F32)
            nc.sync.dma_start(bv_sb[:], bv[:])
            amt_sb = const.tile([128, B * 16], F32)
            nc.sync.dma_start(amt_sb[:], amt[:])
            cmask_sb = const.tile([128, 4 * 512], F32)
            nc.sync.dma_start(cmask_sb[:], cmask_dram[:])
            ident_sb = const.tile([128, 128], F32R)
            nc.gpsimd.dma_start(ident_sb[:], ident_dram[:])
            ones_sb = const.tile([128, 1], F32R)
            nc.gpsimd.dma_start(ones_sb[:], ones_dram[:])

            Identity = mybir.ActivationFunctionType.Identity
            Exp = mybir.ActivationFunctionType.Exp

            for b in range(B):
                # --- phase A: QKV projections for batch b ---
                xts = []
                for kk in range(NKT):
                    xk = xpool.tile([128, T], F32R, name=f"xk{kk}", tag=f"xk{kk}")
                    nc.gpsimd.dma_start(
                        xk[:], xt[kk * 128 : (kk + 1) * 128, b * T : (b + 1) * T]
                    )
                    xts.append(xk)

                qt_sb = qkv.tile([128, T], F32R, name="qt_sb", tag="qt")
                kt_sb = qkv.tile([128, T], F32R, name="kt_sb", tag="kt")
                vt_sb = qkv.tile([128, T], F32R, name="vt_sb", tag="vt")
                for p, (dest, bias) in enumerate(
                    ((qt_sb, bq_sb), (kt_sb, bk_sb), (vt_sb, bv_sb))
                ):
                    for n in range(NIB):
                        ps = psA.tile([128, 512], F32, name="psa", tag="psa")
                        for kk in range(NKT):
                            nc.tensor.matmul(
                                ps[:],
                                wt_sb[:, (p * NKT + kk) * 128 : (p * NKT + kk + 1) * 128],
                                xts[kk][:, n * 512 : (n + 1) * 512],
                                start=(kk == 0),
                                stop=(kk == NKT - 1),
                            )
                        nc.scalar.activation(
                            dest[:, n * 512 : (n + 1) * 512], ps[:], Identity,
                            bias=bias[:, 0:1],
                        )

                # --- V^T -> V_aug (natural layout + ones column) ---
                vas = {}
                for jt in range(NJT):
                    pst = psT.tile([128, 128], F32R, name="pst", tag="pst")
                    nc.tensor.transpose(
                        pst[:], vt_sb[:, jt * 128 : (jt + 1) * 128], ident_sb[:]
                    )
                    for h in range(HPC):
                        va = vapool.tile(
                            [128, 65], F32R, name=f"va{jt}_{h}", tag=f"va{jt}_{h}"
                        )
                        nc.vector.tensor_copy(
                            va[:, 0:64], pst[:, h * 64 : (h + 1) * 64]
                        )
                        nc.vector.tensor_copy(va[:, 64:65], ones_sb[:])
                        vas[(jt, h)] = va

                # --- phase B: attention ---
                for h in range(HPC):
                    qt_h = qt_sb[h * 64 : (h + 1) * 64, :]
                    kt_h = kt_sb[h * 64 : (h + 1) * 64, :]
                    for ib in range(NIB):
                        pso = psO.tile([65, 512], F32, name="pso", tag="pso")
                        njt = 4 * (ib + 1)
                        pts = [None] * njt

                        def emit_s(jt):
                            pss = psS.tile([128, 512], F32, name="pss", tag="pss")
                            nc.tensor.matmul(
                                pss[:],
                                kt_h[:, jt * 128 : (jt + 1) * 128],
                                qt_h[:, ib * 512 : (ib + 1) * 512],
                                start=True,
                                stop=True,
                            )
                            pt = ppool.tile([128, 512], F32R, name="pt", tag="pt")
                            bias_col = amt_sb[:, b * 16 + jt : b * 16 + jt + 1]
                            v = jt - 4 * ib
                            if v >= 0:
                                nc.vector.tensor_add(
                                    pt[:], pss[:], cmask_sb[:, v * 512 : (v + 1) * 512]
                                )
                                nc.scalar.activation(
                                    pt[:], pt[:], Exp, bias=bias_col, scale=0.125
                                )
                            else:
                                nc.scalar.activation(
                                    pt[:], pss[:], Exp, bias=bias_col, scale=0.125
                                )
                            pts[jt] = pt

                        # software-pipeline: S/exp one step ahead of PV
                        emit_s(0)
                        for jt in range(njt):
                            if jt + 1 < njt:
                                emit_s(jt + 1)
                            nc.tensor.matmul(
                                pso[:],
                                vas[(jt, h)][:],
                                pts[jt][:],
                                start=(jt == 0),
                                stop=(jt == njt - 1),
                            )
                        # --- normalize + store ---
                        d_sb = npool.tile([1, 512], F32, name="d_sb", tag="d")
                        nc.scalar.copy(d_sb[:], pso[64:65, :])
                        r_sb = npool.tile([1, 512], F32, name="r_sb", tag="r")
                        nc.vector.reciprocal(r_sb[:], d_sb[:])
                        rb = npool.tile([64, 512], F32, name="rb", tag="rb")
                        nc.gpsimd.partition_broadcast(rb[:], r_sb[:])
                        osb = opool.tile([64, 512], F32, name="osb", tag="osb")
                        nc.vector.tensor_mul(osb[:], pso[0:64, :], rb[:])
                        nc.sync.dma_start(
                            out[b, h, :, ib * 512 : (ib + 1) * 512], osb[:]
                        )

    nc.compile()
    return nc


def kernel(hidden_states, attention_mask, Wq, bq, Wk, bk, Wv, bv):
    from concourse.bass_utils import run_bass_kernel_spmd

    if "nc" not in _CACHE:
        _CACHE["nc"] = _build()
    nc = _CACHE["nc"]

    hidden_states = np.asarray(hidden_states, dtype=np.float32)
    attention_mask = np.asarray(attention_mask, dtype=np.float32)
    Wq, Wk, Wv = (np.asarray(w, dtype=np.float32) for w in (Wq, Wk, Wv))
    bq, bk, bv = (np.asarray(v, dtype=np.float32) for v in (bq, bk, bv))

    xt = np.ascontiguousarray(hidden_states.reshape(BT, H).T)
    # amt[:, b*16+jt] = attention_mask[b, 0, 0, jt*128:(jt+1)*128]
    amt = np.ascontiguousarray(
        attention_mask.reshape(B, 16, 128).transpose(2, 0, 1).reshape(128, B * 16)
    )

    in_maps = []
    for c in range(NCORES):
        sl = slice(c * HPC * HD, (c + 1) * HPC * HD)  # this core's 128 head dims
        in_maps.append(
            {
                "xt": xt,
                "wqt": np.ascontiguousarray(Wq[sl, :].T),
                "wkt": np.ascontiguousarray(Wk[sl, :].T),
                "wvt": np.ascontiguousarray(Wv[sl, :].T),
                "bq": np.ascontiguousarray(bq[sl, None]),
                "bk": np.ascontiguousarray(bk[sl, None]),
                "bv": np.ascontiguousarray(bv[sl, None]),
                "amt": amt,
            }
        )

    res = run_bass_kernel_spmd(nc, in_maps, core_ids=list(range(NCORES)))

    full = np.empty((B, NH, T, HD), dtype=np.float32)
    for c in range(NCORES):
        o = res.results[c]["out"]  # [B, HPC, HD, T]
        full[:, c * HPC : (c + 1) * HPC] = o.transpose(0, 1, 3, 2)
    return full


# revision 6
# speedup vs baseline: 19748.2376x; 19748.2376x over previous
"""Causal self-attention (B=4, T=2048, H=1024, NH=16, HD=64) on 8 trn2 cores.

Sharding: tensor-parallel over heads — core c computes heads 2c and 2c+1 for
all batches. Q/K/V weights are column-sharded by head (host slices + pre-
transposes them); hidden_states is pre-transposed on host to X^T [H, B*T] so
the contraction dim (H) lands on SBUF partitions for every matmul.

Per-core dataflow (all matmuls fp32r = full-rate PE with ~1e-4 rounding):
  phase A (per batch b):
    X^T_b k-tiles [128, 2048] -> Q^T, K^T, V^T [128=2*64 d, 2048 t] (bias via
    ACT); V^T -> PE-transpose -> V_aug[jt] = [V_nat[j, d_head] | ones]
    (the ones column makes the PV matmul emit the softmax denominator as
    output row 64; ones are written once at startup, tiles are persistent)
  phase B (per b, query block ib of 512, key tile jt <= diag):
    S^T[j,i] for BOTH heads into one wide PSUM [128, 1024]
    (h0 at cols 0:512 / PE rows 0-63, h1 at cols 512:1024 / PE rows 64-127 —
    adjacent row-group matmuls run concurrently in the array);
    diagonal-straddling tiles restrict i to the unmasked range (N=512-128v)
    P^T = exp(S^T/8 + causal + attn_mask[j]) — one wide ACT op per jt
    O^T[65, 512] += V_aug[jt,h].T @ P^T_h   (row 64 = denominator)
    normalize: DVE recip(denoms) -> one gpsimd partition_broadcast -> DVE mul
  output per core: O^T [4, 2, 64, 2048]; host transposes to [4, 2, 2048, 64]
  and concatenates heads.
"""

import numpy as np

B, T, H, NH = 4, 2048, 1024, 16
HD = H // NH  # 64
NCORES = 8
HPC = NH // NCORES  # heads per core = 2
BT = B * T

_CACHE = {}


def _build(reps=1):
    import contextlib
    from contextlib import ExitStack

    import concourse.mybir as mybir
    import concourse.tile as tile
    from concourse import bacc

    F32 = mybir.dt.float32
    F32R = mybir.dt.float32r

    nc = bacc.Bacc("TRN2", target_bir_lowering=False, num_devices=NCORES)

    # fp32r params: numpy fp32 bits, PE rounds on read; lets HWDGE (sync) DMA
    # them without the gpsimd cast path.
    xt = nc.declare_dram_parameter("xt", [H, BT], F32R, isOutput=False)
    wqt = nc.declare_dram_parameter("wqt", [H, 128], F32R, isOutput=False)
    wkt = nc.declare_dram_parameter("wkt", [H, 128], F32R, isOutput=False)
    wvt = nc.declare_dram_parameter("wvt", [H, 128], F32R, isOutput=False)
    bq = nc.declare_dram_parameter("bq", [128, 1], F32, isOutput=False)
    bk = nc.declare_dram_parameter("bk", [128, 1], F32, isOutput=False)
    bv = nc.declare_dram_parameter("bv", [128, 1], F32, isOutput=False)
    # amt[:, b*16+jt] = attention_mask[b, 0, 0, jt*128:(jt+1)*128]
    amt = nc.declare_dram_parameter("amt", [128, B * 16], F32, isOutput=False)
    out = nc.declare_dram_parameter("out", [B, HPC, HD, T], F32, isOutput=True)

    # Triangular causal mask (two-head-wide): masks j > i within a 512 block,
    # duplicated at cols 512:1024 for the second head. Diagonal-straddling
    # tiles with offset v use cols [0:512-128v] of the first triangle.
    jj = np.arange(128)[:, None]
    ii = np.arange(512)[None, :]
    tri = np.where(jj > ii, -1e9, 0.0).astype(np.float32)
    cmask_dram = nc.inline_tensor(
        np.concatenate([tri, tri], axis=1), name="cmask"
    )
    ident_dram = nc.inline_tensor(np.eye(128, dtype=np.float32), name="ident")
    ones_dram = nc.inline_tensor(np.ones((128, 1), dtype=np.float32), name="ones")

    NKT = H // 128  # 8 contraction tiles
    NIB = T // 512  # 4 query blocks
    NJT = T // 128  # 16 key tiles

    with tile.TileContext(nc) as tc:
        with ExitStack() as ctx:
            const = ctx.enter_context(tc.tile_pool(name="const", bufs=1))
            xpool = ctx.enter_context(tc.tile_pool(name="xpool", bufs=1))
            qkv = ctx.enter_context(tc.tile_pool(name="qkv", bufs=2))
            vapool = ctx.enter_context(tc.tile_pool(name="vapool", bufs=1))
            ppool = ctx.enter_context(tc.tile_pool(name="ppool", bufs=4))
            opool = ctx.enter_context(tc.tile_pool(name="opool", bufs=4))
            npool = ctx.enter_context(tc.tile_pool(name="npool", bufs=4))
            # PSUM: wide 2-bank tag (proj/transpose/S) x2 + two 1-bank O
            # accumulators x2 = 8 banks.
            psW = ctx.enter_context(tc.tile_pool(name="psW", bufs=2, space="PSUM"))
            psO = ctx.enter_context(tc.tile_pool(name="psO", bufs=2, space="PSUM"))

            # --- constants / weights ---
            wt_sb = const.tile([128, 3 * H], F32R)
            for p, w in enumerate((wqt, wkt, wvt)):
                for kk in range(NKT):
                    nc.sync.dma_start(
                        wt_sb[:, (p * NKT + kk) * 128 : (p * NKT + kk + 1) * 128],
                        w[kk * 128 : (kk + 1) * 128, :],
                    )
            bq_sb = const.tile([128, 1], F32)
            nc.sync.dma_start(bq_sb[:], bq[:])
            bk_sb = const.tile([128, 1], F32)
            nc.sync.dma_start(bk_sb[:], bk[:])
            bv_sb = const.tile([128, 1], F32)
            nc.sync.dma_start(bv_sb[:], bv[:])
            amt_sb = const.tile([128, B * 16], F32)
            nc.sync.dma_start(amt_sb[:], amt[:])
            cmask_sb = const.tile([128, 1024], F32)
            nc.sync.dma_start(cmask_sb[:], cmask_dram[:])
            ident_sb = const.tile([128, 128], F32R)
            nc.gpsimd.dma_start(ident_sb[:], ident_dram[:])
            ones_sb = const.tile([128, 1], F32R)
            nc.gpsimd.dma_start(ones_sb[:], ones_dram[:])

            # persistent V_aug tiles; ones column written once here
            vas = {}
            for jt in range(NJT):
                for h in range(HPC):
                    va = vapool.tile(
                        [128, 65], F32R, name=f"va{jt}_{h}", tag=f"va{jt}_{h}"
                    )
                    nc.vector.tensor_copy(va[:, 64:65], ones_sb[:])
                    vas[(jt, h)] = va

            Identity = mybir.ActivationFunctionType.Identity
            Exp = mybir.ActivationFunctionType.Exp

            loop_ctx = tc.For_i(0, reps, 1) if reps > 1 else contextlib.nullcontext()
            with loop_ctx:
                for b in range(B):
                    # --- phase A: QKV projections for batch b ---
                    xts = []
                    for kk in range(NKT):
                        xk = xpool.tile([128, T], F32R, name=f"xk{kk}", tag=f"xk{kk}")
                        nc.sync.dma_start(
                            xk[:], xt[kk * 128 : (kk + 1) * 128, b * T : (b + 1) * T]
                        )
                        xts.append(xk)

                    qt_sb = qkv.tile([128, T], F32R, name="qt_sb", tag="qt")
                    kt_sb = qkv.tile([128, T], F32R, name="kt_sb", tag="kt")
                    vt_sb = qkv.tile([128, T], F32R, name="vt_sb", tag="vt")
                    for p, (dest, bias) in enumerate(
                        ((qt_sb, bq_sb), (kt_sb, bk_sb), (vt_sb, bv_sb))
                    ):
                        for nw in range(NIB // 2):  # two 512-blocks per wide psum
                            ps = psW.tile([128, 1024], F32, name="psw", tag="wide")
                            for half in range(2):
                                n = nw * 2 + half
                                for kk in range(NKT):
                                    nc.tensor.matmul(
                                        ps[:, half * 512 : (half + 1) * 512],
                                        wt_sb[
                                            :,
                                            (p * NKT + kk) * 128
                                            : (p * NKT + kk + 1) * 128,
                                        ],
                                        xts[kk][:, n * 512 : (n + 1) * 512],
                                        start=(kk == 0),
                                        stop=(kk == NKT - 1),
                                    )
                            nc.scalar.activation(
                                dest[:, nw * 1024 : (nw + 1) * 1024], ps[:], Identity,
                                bias=bias[:, 0:1],
                            )

                    # --- V^T -> V_aug (natural layout) ---
                    for jt in range(NJT):
                        pst = psW.tile([128, 128], F32R, name="pst", tag="wide")
                        nc.tensor.transpose(
                            pst[:], vt_sb[:, jt * 128 : (jt + 1) * 128], ident_sb[:]
                        )
                        for h in range(HPC):
                            nc.vector.tensor_copy(
                                vas[(jt, h)][:, 0:64], pst[:, h * 64 : (h + 1) * 64]
                            )

                    # --- phase B: attention, both heads interleaved ---
                    for ib in range(NIB):
                        njt = 4 * (ib + 1)
                        pso = [
                            psO.tile([65, 512], F32, name=f"pso{h}", tag=f"pso{h}")
                            for h in range(HPC)
                        ]
                        pts = [None] * njt

                        def emit_s(jt):
                            # diagonal-straddling tiles only need i >= j:
                            # column offset off = 128*v within the 512 block
                            v = jt - 4 * ib
                            off = 128 * v if v > 0 else 0
                            nn = 512 - off
                            pss = psW.tile([128, 1024], F32, name="pss", tag="wide")
                            for h in range(HPC):
                                nc.tensor.matmul(
                                    pss[:, h * 512 + off : (h + 1) * 512],
                                    kt_sb[
                                        h * 64 : (h + 1) * 64,
                                        jt * 128 : (jt + 1) * 128,
                                    ],
                                    qt_sb[
                                        h * 64 : (h + 1) * 64,
                                        ib * 512 + off : (ib + 1) * 512,
                                    ],
                                    start=True,
                                    stop=True,
                                )
                            pt = ppool.tile([128, 1024], F32R, name="pt", tag="pt")
                            bias_col = amt_sb[:, b * 16 + jt : b * 16 + jt + 1]
                            if v == 0:
                                # one wide masked exp covering both heads
                                nc.vector.tensor_add(pt[:], pss[:], cmask_sb[:])
                                nc.scalar.activation(
                                    pt[:], pt[:], Exp, bias=bias_col, scale=0.125
                                )
                            elif v > 0:
                                for h in range(HPC):
                                    sl = slice(h * 512 + off, (h + 1) * 512)
                                    nc.vector.tensor_add(
                                        pt[:, sl], pss[:, sl], cmask_sb[:, 0:nn]
                                    )
                                    nc.scalar.activation(
                                        pt[:, sl], pt[:, sl], Exp,
                                        bias=bias_col, scale=0.125,
                                    )
                            else:
                                nc.scalar.activation(
                                    pt[:], pss[:], Exp, bias=bias_col, scale=0.125
                                )
                            pts[jt] = (pt, off)

                        # software-pipeline: S/exp two steps ahead of PV
                        emit_s(0)
                        if njt > 1:
                            emit_s(1)
                        for jt in range(njt):
                            if jt + 2 < njt:
                                emit_s(jt + 2)
                            pt, off = pts[jt]
                            for h in range(HPC):
                                nc.tensor.matmul(
                                    pso[h][:, off:512],
                                    vas[(jt, h)][:],
                                    pt[:, h * 512 + off : (h + 1) * 512],
                                    start=(jt == 0),
                                    stop=(jt == njt - 1),
                                )

                        # --- normalize + store (heads merged) ---
                        r_sb = npool.tile([1, 1024], F32, name="r_sb", tag="r")
                        for h in range(HPC):
                            nc.vector.reciprocal(
                                r_sb[:, h * 512 : (h + 1) * 512], pso[h][64:65, :]
                            )
                        rb = npool.tile([64, 1024], F32, name="rb", tag="rb")
                        nc.gpsimd.partition_broadcast(rb[:], r_sb[:])
                        for h in range(HPC):
                            osb = opool.tile([64, 512], F32, name="osb", tag="osb")
                            nc.vector.tensor_mul(
                                osb[:], pso[h][0:64, :], rb[:, h * 512 : (h + 1) * 512]
                            )
                            nc.sync.dma_start(
                                out[b, h, :, ib * 512 : (ib + 1) * 512], osb[:]
                            )

    nc.compile()
    return nc


def kernel(hidden_states, attention_mask, Wq, bq, Wk, bk, Wv, bv):
    from concourse.bass_utils import run_bass_kernel_spmd

    if "nc" not in _CACHE:
        _CACHE["nc"] = _build()
    nc = _CACHE["nc"]

    hidden_states = np.asarray(hidden_states, dtype=np.float32)
    attention_mask = np.asarray(attention_mask, dtype=np.float32)
    Wq, Wk, Wv = (np.asarray(w, dtype=np.float32) for w in (Wq, Wk, Wv))
    bq, bk, bv = (np.asarray(v, dtype=np.float32) for v in (bq, bk, bv))

    xt = np.ascontiguousarray(hidden_states.reshape(BT, H).T)
    # amt[:, b*16+jt] = attention_mask[b, 0, 0, jt*128:(jt+1)*128]
    amt = np.ascontiguousarray(
        attention_mask.reshape(B, 16, 128).transpose(2, 0, 1).reshape(128, B * 16)
    )

    in_maps = []
    for c in range(NCORES):
        sl = slice(c * HPC * HD, (c + 1) * HPC * HD)  # this core's 128 head dims
        in_maps.append(
            {
                "xt": xt,
                "wqt": np.ascontiguousarray(Wq[sl, :].T),
                "wkt": np.ascontiguousarray(Wk[sl, :].T),
                "wvt": np.ascontiguousarray(Wv[sl, :].T),
                "bq": np.ascontiguousarray(bq[sl, None]),
                "bk": np.ascontiguousarray(bk[sl, None]),
                "bv": np.ascontiguousarray(bv[sl, None]),
                "amt": amt,
            }
        )

    res = run_bass_kernel_spmd(nc, in_maps, core_ids=list(range(NCORES)))

    full = np.empty((B, NH, T, HD), dtype=np.float32)
    for c in range(NCORES):
        o = res.results[c]["out"]  # [B, HPC, HD, T]
        full[:, c * HPC : (c + 1) * HPC] = o.transpose(0, 1, 3, 2)
    return full
